# revision 6
# baseline (speedup 1.0000x reference)
"""Trainium2 Bass kernel for a dense transformer block (B=4,T=2048,C=1024,H=16).

Sharding: 8 cores, zero collectives. Core i handles batch i//2 and a
balanced half of the query tokens (i%2==0 -> chunks [0:512)+[1536:2048),
i%2==1 -> [512:1024)+[1024:1536)). All sharding is done on the host; the
device program is identical on every core (SPMD), only input data differs.

Per-core dataflow (tokens-on-free-axis for all matmul operands):
  LN1 (bn_stats, [tok,C] layout) -> h1 bf16 -> DRAM bounce -> DMA-transpose
  -> h1T [C, tok].  QKV in bf16: kT/qT stacked [H*64, tok] (2 heads per
  128-partition tile), V natural [tok, H*65] with a fused ones column so the
  PV matmul also produces the softmax denominator.  Scores are computed
  pre-transposed sT[tk, tq] = K Q^T so softmax needs no transpose of P and
  no max subtraction (scores are O(1)); exp runs on ACT straight from PSUM
  with the 1/sqrt(hd) scale fused.  Causality at 128-tile granularity with
  per-core host-built multiplicative masks (uniform program, per-core data).
  PV accumulates over tk tiles in PSUM; normalization = one reciprocal of
  the denominator row + gpsimd partition_broadcast + one multiply.
  proj and both FFN matmuls run in float32r (full-rate fp32).
"""

import sys
import numpy as np

for _p in ("/opt/trn_rl_repo", "/root/.axon_site/_ro/trn_rl_repo"):
    if _p not in sys.path:
        sys.path.append(_p)

import ml_dtypes  # noqa: E402
import concourse.bass as bass  # noqa: E402
import concourse.bacc as bacc  # noqa: E402
import concourse.tile as tile  # noqa: E402
from concourse import mybir  # noqa: E402
from concourse.bass_utils import run_bass_kernel_spmd  # noqa: E402
from concourse.masks import make_identity  # noqa: E402

B, T, C, H, HD = 4, 2048, 1024, 16, 64
NCORES = 8
EPS = 1e-5
F32 = mybir.dt.float32
F32R = mybir.dt.float32r
BF16 = mybir.dt.bfloat16
AF = mybir.ActivationFunctionType
ALU = mybir.AluOpType

_CACHE = {}


def _emit_body(nc, tc, io, ln1_triv, ln2_triv):
    # ---------------- long-lived pools ----------------
    def pool(name, bufs, space="SBUF"):
        cm = tc.tile_pool(name=name, bufs=bufs, space=space)
        p = cm.__enter__()
        return cm, p

    cm_singles, singles = pool("singles", 1)
    cm_ln, ln_pool = pool("ln", 2)
    cm_stat, stat_pool = pool("stat", 4)
    cm_small, small = pool("small", 2)
    cm_dram, dram = pool("dram", 1, "DRAM")
    cm_psg, ps_gen = pool("ps_gen", 2, "PSUM")

    eps_t = singles.tile([128, 1], F32, name="eps")
    nc.vector.memset(eps_t, EPS)

    def bcast_ap(dram_ap):
        # [1024] dram vector -> [128,1024] partition-broadcast AP
        return bass.AP(
            tensor=dram_ap.tensor,
            offset=dram_ap.offset,
            ap=[[0, 128]] + list(dram_ap.ap),
        )

    g1_sb = bb1_sb = g2_sb = bb2_sb = None
    if not ln1_triv:
        g1_sb = singles.tile([128, 1024], F32, name="g1_sb")
        nc.gpsimd.dma_start(out=g1_sb, in_=bcast_ap(io["ln1_g"]))
        bb1_sb = singles.tile([128, 1024], F32, name="bb1_sb")
        nc.gpsimd.dma_start(out=bb1_sb, in_=bcast_ap(io["ln1_b"]))

    # ---------------- LayerNorm helpers ----------------
    def ln_apply(xt, out_ap, trivial, g_sb, b_sb):
        st = stat_pool.tile([128, 2, 6], F32, tag="bnst", name="bnst")
        for sg in range(2):
            nc.vector.bn_stats(out=st[:, sg, :], in_=xt[:, sg * 512:(sg + 1) * 512])
        mv = stat_pool.tile([128, 2], F32, tag="bnmv", name="bnmv")
        nc.vector.bn_aggr(out=mv, in_=st)
        std = stat_pool.tile([128, 1], F32, tag="bnsd", name="bnsd")
        nc.scalar.activation(out=std, in_=mv[:, 1:2], func=AF.Sqrt, bias=eps_t,
                             scale=1.0)
        rstd = stat_pool.tile([128, 1], F32, tag="bnrs", name="bnrs")
        nc.vector.reciprocal(out=rstd, in_=std)
        if trivial:
            nc.vector.tensor_scalar(
                out=out_ap, in0=xt, scalar1=mv[:, 0:1], scalar2=rstd,
                op0=ALU.subtract, op1=ALU.mult)
        else:
            tmp = ln_pool.tile([128, 1024], F32, tag="lnx", name="lntmp")
            nc.vector.tensor_scalar(
                out=tmp, in0=xt, scalar1=mv[:, 0:1], scalar2=rstd,
                op0=ALU.subtract, op1=ALU.mult)
            nc.vector.tensor_mul(out=tmp, in0=tmp, in1=g_sb)
            nc.vector.tensor_add(out=out_ap, in0=tmp, in1=b_sb)

    def ln_tile(x_src_rows, out_ap, trivial, g_sb, b_sb):
        xt = ln_pool.tile([128, 1024], F32, tag="lnx", name="lnx")
        nc.sync.dma_start(out=xt, in_=x_src_rows)
        ln_apply(xt, out_ap, trivial, g_sb, b_sb)

    # result pools that outlive the QKV scratch (opened early: LIFO order)
    cm_kt, kt_pool = pool("kt", 8)
    cm_v, v_pool = pool("v", 16)
    cm_qt, qt_pool = pool("qt", 8)
    kT = [kt_pool.tile([128, 2048], BF16, tag="kt", name="kt") for _ in range(8)]
    Vt = [v_pool.tile([128, 16, 65], BF16, tag="vt", name="vt") for _ in range(16)]
    qT = [qt_pool.tile([128, 1024], BF16, tag="qt", name="qt") for _ in range(8)]

    # ---------------- Phase 1: LN1 -> h1 (bf16) -> DRAM -> h1T, h1Th -------
    cm_h1t, h1t_pool = pool("h1t", 8)
    cm_h1th, h1th_pool = pool("h1th", 8)
    cm_h1, h1_pool = pool("h1", 2)

    h1d = dram.tile([2048, 1024], BF16, name="h1d")
    for t in range(16):
        ht = h1_pool.tile([128, 1024], BF16, tag="h1", name="h1")
        ln_tile(io["x_full"][t * 128:(t + 1) * 128, :], ht, ln1_triv, g1_sb, bb1_sb)
        nc.sync.dma_start(out=h1d[t * 128:(t + 1) * 128, :], in_=ht)
    h1T = []
    for c in range(8):
        tt = h1t_pool.tile([128, 2048], BF16, tag="h1t", name="h1t")
        nc.sync.dma_start(out=tt, in_=h1d[:, c * 128:(c + 1) * 128], transpose=True)
        h1T.append(tt)

    h1dh = dram.tile([1024, 1024], BF16, name="h1dh")
    for t in range(8):
        ht = h1_pool.tile([128, 1024], BF16, tag="h1", name="h1h")
        ln_tile(io["x_half"][t * 128:(t + 1) * 128, :], ht, ln1_triv, g1_sb, bb1_sb)
        nc.sync.dma_start(out=h1dh[t * 128:(t + 1) * 128, :], in_=ht)
    h1Th = []
    for c in range(8):
        tt = h1th_pool.tile([128, 1024], BF16, tag="h1th", name="h1th")
        nc.sync.dma_start(out=tt, in_=h1dh[:, c * 128:(c + 1) * 128], transpose=True)
        h1Th.append(tt)
    cm_h1.__exit__(None, None, None)

    # ---------------- Phase 2: QKV ----------------
    # kT[p]: heads 2p (rows 0:64) / 2p+1 (rows 64:128), cols = all 2048 keys
    cm_wk, wk_pool = pool("wk", 8)
    wk_sb = []
    for c in range(8):
        t = wk_pool.tile([128, 1024], BF16, tag="wk", name="wk")
        nc.sync.dma_start(out=t, in_=io["wqk"][c * 128:(c + 1) * 128, 1024:2048])
        wk_sb.append(t)
    for p in range(8):
        for n in range(4):
            ps = ps_gen.tile([128, 512], F32, tag="g", name="psk")
            for c in range(8):
                nc.tensor.matmul(
                    out=ps, lhsT=wk_sb[c][:, p * 128:(p + 1) * 128],
                    rhs=h1T[c][:, n * 512:(n + 1) * 512],
                    start=(c == 0), stop=(c == 7))
            nc.vector.tensor_copy(out=kT[p][:, n * 512:(n + 1) * 512], in_=ps)
    cm_wk.__exit__(None, None, None)

    # V_aug[t]: [128 tok, 16 heads, 65] bf16 (col 64 = ones)
    cm_wv, wv_pool = pool("wv", 8)
    wv_sb = []
    for c in range(8):
        t = wv_pool.tile([128, 1024], BF16, tag="wv", name="wv")
        nc.sync.dma_start(out=t, in_=io["wv"][c * 128:(c + 1) * 128, :])
        wv_sb.append(t)
    for t in range(16):
        for n in range(2):
            ps = ps_gen.tile([128, 512], F32, tag="g", name="psv")
            for c in range(8):
                nc.tensor.matmul(
                    out=ps, lhsT=h1T[c][:, t * 128:(t + 1) * 128],
                    rhs=wv_sb[c][:, n * 512:(n + 1) * 512],
                    start=(c == 0), stop=(c == 7))
            nc.vector.tensor_copy(
                out=Vt[t][:, n * 8:(n + 1) * 8, 0:64],
                in_=ps.rearrange("p (h d) -> p h d", d=64))
        nc.vector.memset(Vt[t][:, :, 64:65], 1.0)
    cm_wv.__exit__(None, None, None)

    # qT[p]: [128(h2,d), 1024 my queries]
    cm_wq, wq_pool = pool("wq", 8)
    wq_sb = []
    for c in range(8):
        t = wq_pool.tile([128, 1024], BF16, tag="wq", name="wq")
        nc.sync.dma_start(out=t, in_=io["wqk"][c * 128:(c + 1) * 128, 0:1024])
        wq_sb.append(t)
    for p in range(8):
        for n in range(2):
            ps = ps_gen.tile([128, 512], F32, tag="g", name="psq")
            for c in range(8):
                nc.tensor.matmul(
                    out=ps, lhsT=wq_sb[c][:, p * 128:(p + 1) * 128],
                    rhs=h1Th[c][:, n * 512:(n + 1) * 512],
                    start=(c == 0), stop=(c == 7))
            nc.vector.tensor_copy(out=qT[p][:, n * 512:(n + 1) * 512], in_=ps)
    cm_wq.__exit__(None, None, None)
    cm_h1th.__exit__(None, None, None)
    cm_h1t.__exit__(None, None, None)

    # ---------------- Phase 3: attention ----------------
    cm_masks, masks_pool = pool("masks", 1)
    masks_sb = masks_pool.tile([128, 16, 512], BF16, name="masks_sb")
    nc.sync.dma_start(out=masks_sb, in_=io["masks"])

    cm_pt, pt_pool = pool("pt", 3)
    cm_ast, ast_pool = pool("attst", 4)
    cm_pssc, ps_sc = pool("ps_sc", 2, "PSUM")
    cm_pspv, ps_pv = pool("ps_pv", 2, "PSUM")

    attd = dram.tile([1024, 1024], F32, name="attd")  # [c=h*64+d, my tq]
    SCALE = HD ** -0.5

    for s in range(2):          # slot 0: 8 tk tiles / slot 1: 16 tk tiles
        ntk = 8 if s == 0 else 16
        qc = s * 512
        for hp in range(8):
            pva = [ps_pv.tile([128, 512], F32, tag="pv", name="pv")
                   for _ in range(2)]
            for tkt in range(ntk):
                ps = ps_sc.tile([128, 2, 512], F32, tag="sc", name="sc")
                for e in range(2):
                    nc.tensor.matmul(
                        out=ps[:, e, :],
                        lhsT=kT[hp][e * 64:(e + 1) * 64,
                                    tkt * 128:(tkt + 1) * 128],
                        rhs=qT[hp][e * 64:(e + 1) * 64, qc:qc + 512],
                        start=True, stop=True)
                pt = pt_pool.tile([128, 2, 512], BF16, tag="pt", name="pt")
                nc.scalar.activation(
                    out=pt.rearrange("p a b -> p (a b)"),
                    in_=ps.rearrange("p a b -> p (a b)"),
                    func=AF.Exp, scale=SCALE)
                if (s == 0) or (tkt >= 8):
                    for e in range(2):
                        nc.vector.tensor_mul(
                            out=pt[:, e, :], in0=pt[:, e, :],
                            in1=masks_sb[:, tkt, :])
                for e in range(2):
                    nc.tensor.matmul(
                        out=pva[e][0:65, :],
                        lhsT=Vt[tkt][:, 2 * hp + e, :],
                        rhs=pt[:, e, :],
                        start=(tkt == 0), stop=(tkt == ntk - 1))
            for e in range(2):
                rec = small.tile([1, 512], F32, tag="rec", name="rec")
                nc.vector.reciprocal(out=rec, in_=pva[e][64:65, :])
                bc = small.tile([64, 512], F32, tag="bc", name="bc")
                nc.gpsimd.partition_broadcast(out_ap=bc, in_ap=rec)
                ast = ast_pool.tile([64, 512], F32, tag="ast", name="ast")
                nc.vector.tensor_mul(out=ast, in0=pva[e][0:64, :], in1=bc)
                nc.sync.dma_start(
                    out=attd[hp * 128 + e * 64:hp * 128 + (e + 1) * 64,
                             qc:qc + 512],
                    in_=ast)
    cm_pspv.__exit__(None, None, None)
    cm_pssc.__exit__(None, None, None)
    cm_ast.__exit__(None, None, None)
    cm_pt.__exit__(None, None, None)
    cm_masks.__exit__(None, None, None)
    cm_qt.__exit__(None, None, None)
    cm_v.__exit__(None, None, None)
    cm_kt.__exit__(None, None, None)

    # ---------------- late constants ----------------
    cm_late, late = pool("late", 1)
    ident = late.tile([128, 128], F32, name="ident")
    make_identity(nc, ident)
    b1t_sb = late.tile([128, 32], F32, name="b1t_sb")
    nc.sync.dma_start(out=b1t_sb, in_=io["b1t"])
    bproj_sb = late.tile([128, 1024], F32, name="bproj_sb")
    nc.gpsimd.dma_start(out=bproj_sb, in_=bcast_ap(io["b_proj"]))
    b2_sb = late.tile([128, 1024], F32, name="b2_sb")
    nc.gpsimd.dma_start(out=b2_sb, in_=bcast_ap(io["b2"]))
    if not ln2_triv:
        g2_sb = late.tile([128, 1024], F32, name="g2_sb")
        nc.gpsimd.dma_start(out=g2_sb, in_=bcast_ap(io["ln2_g"]))
        bb2_sb = late.tile([128, 1024], F32, name="bb2_sb")
        nc.gpsimd.dma_start(out=bb2_sb, in_=bcast_ap(io["ln2_b"]))

    # ---------------- Phase 4: proj + residual -> x2 ----------------
    cm_x2, x2_pool = pool("x2", 8)
    cm_h2t, h2t_pool = pool("h2t", 8)
    cm_wp, wp_pool = pool("wproj", 8)
    cm_att, att_pool = pool("attls", 16)
    wproj_sb = []
    for c in range(8):
        t = wp_pool.tile([128, 1024], F32R, tag="wproj", name="wproj")
        nc.sync.dma_start(
            out=t, in_=io["w_proj"][c * 128:(c + 1) * 128, :].bitcast(F32R))
        wproj_sb.append(t)

    x2 = [x2_pool.tile([128, 1024], F32, tag="x2", name="x2") for _ in range(8)]
    for t in range(8):
        acts = []
        for c in range(8):
            a = att_pool.tile([128, 128], F32R, tag="attls", name="attls")
            nc.sync.dma_start(
                out=a,
                in_=attd[c * 128:(c + 1) * 128,
                         t * 128:(t + 1) * 128].bitcast(F32R))
            acts.append(a)
        xh = ln_pool.tile([128, 1024], F32, tag="lnx", name="xh2")
        nc.sync.dma_start(out=xh, in_=io["x_half"][t * 128:(t + 1) * 128, :])
        for n in range(2):
            ps = ps_gen.tile([128, 512], F32, tag="g", name="psp")
            for c in range(8):
                nc.tensor.matmul(
                    out=ps,
                    lhsT=acts[c],
                    rhs=wproj_sb[c][:, n * 512:(n + 1) * 512],
                    start=(c == 0), stop=(c == 7))
            sl = np.s_[:, n * 512:(n + 1) * 512]
            nc.vector.tensor_add(out=x2[t][sl], in0=ps, in1=xh[sl])
            nc.vector.tensor_add(out=x2[t][sl], in0=x2[t][sl], in1=bproj_sb[sl])
    cm_att.__exit__(None, None, None)
    cm_wp.__exit__(None, None, None)

    # ---------------- Phase 5: LN2 -> h2 -> h2T (PE transpose) --------------
    h2T = [h2t_pool.tile([128, 1024], F32R, tag="h2t", name="h2t")
           for _ in range(8)]
    for t in range(8):
        h2 = ln_pool.tile([128, 1024], F32, tag="lnx", name="h2")
        ln_apply(x2[t], h2, ln2_triv, g2_sb, bb2_sb)
        for c in range(8):
            pst = ps_gen.tile([128, 128], F32, tag="g", name="pst")
            nc.tensor.transpose(out=pst, in_=h2[:, c * 128:(c + 1) * 128],
                                identity=ident)
            nc.vector.tensor_copy(out=h2T[c][:, t * 128:(t + 1) * 128], in_=pst)

    # ---------------- Phase 6: FFN (2 passes x 4 j-blocks) ------------------
    cm_wb, wbig_pool = pool("wbig", 8)
    cm_rl, relu_pool = pool("relu", 1)
    cm_oa, oacc_pool = pool("oacc", 4)
    for pas in range(2):
        tok0 = pas * 512
        oacc = [oacc_pool.tile([128, 1024], F32, tag="oacc", name="oacc")
                for _ in range(4)]
        for jb in range(4):
            w1b = [wbig_pool.tile([128, 1024], F32R, tag="wb", name="w1b")
                   for _ in range(8)]
            for c in range(8):
                nc.sync.dma_start(
                    out=w1b[c],
                    in_=io["w1"][c * 128:(c + 1) * 128,
                                 jb * 1024:(jb + 1) * 1024].bitcast(F32R))
            relu_b = relu_pool.tile([128, 8, 512], F32R, tag="rl", name="rl")
            for j in range(8):
                ps = ps_gen.tile([128, 512], F32, tag="g", name="psf1")
                for c in range(8):
                    nc.tensor.matmul(
                        out=ps,
                        lhsT=w1b[c][:, j * 128:(j + 1) * 128],
                        rhs=h2T[c][:, tok0:tok0 + 512],
                        start=(c == 0), stop=(c == 7))
                nc.scalar.activation(
                    out=relu_b[:, j, :], in_=ps, func=AF.Relu,
                    bias=b1t_sb[:, jb * 8 + j:jb * 8 + j + 1], scale=1.0)
            w2b = [wbig_pool.tile([128, 1024], F32R, tag="wb", name="w2b")
                   for _ in range(8)]
            for j in range(8):
                nc.sync.dma_start(
                    out=w2b[j],
                    in_=io["w2"][jb * 1024 + j * 128:
                                 jb * 1024 + (j + 1) * 128, :].bitcast(F32R))
            for tl in range(4):
                for n in range(2):
                    ps = ps_gen.tile([128, 512], F32, tag="g", name="psf2")
                    for j in range(8):
                        nc.tensor.matmul(
                            out=ps,
                            lhsT=relu_b[:, j, tl * 128:(tl + 1) * 128],
                            rhs=w2b[j][:, n * 512:(n + 1) * 512],
                            start=(j == 0), stop=(j == 7))
                    sl = np.s_[:, n * 512:(n + 1) * 512]
                    if jb == 0:
                        nc.vector.tensor_copy(out=oacc[tl][sl], in_=ps)
                    else:
                        nc.vector.tensor_add(out=oacc[tl][sl], in0=oacc[tl][sl],
                                             in1=ps)
        for tl in range(4):
            tg = pas * 4 + tl
            nc.vector.tensor_add(out=oacc[tl], in0=oacc[tl], in1=x2[tg])
            nc.vector.tensor_add(out=oacc[tl], in0=oacc[tl], in1=b2_sb)
            nc.sync.dma_start(out=io["out"][tg * 128:(tg + 1) * 128, :],
                              in_=oacc[tl])

    cm_oa.__exit__(None, None, None)
    cm_rl.__exit__(None, None, None)
    cm_wb.__exit__(None, None, None)
    cm_h2t.__exit__(None, None, None)
    cm_x2.__exit__(None, None, None)
    cm_late.__exit__(None, None, None)
    cm_psg.__exit__(None, None, None)
    cm_dram.__exit__(None, None, None)
    cm_small.__exit__(None, None, None)
    cm_stat.__exit__(None, None, None)
    cm_ln.__exit__(None, None, None)
    cm_singles.__exit__(None, None, None)


def build(ln1_triv=True, ln2_triv=True):
    key = (ln1_triv, ln2_triv)
    if key in _CACHE:
        return _CACHE[key]
    nc = bacc.Bacc("TRN2", target_bir_lowering=False, debug=False,
                   num_devices=NCORES)
    io = {}

    def din(name, shape, dt):
        io[name] = nc.dram_tensor(name, list(shape), dt, kind="ExternalInput").ap()

    din("x_full", (2048, 1024), F32)
    din("x_half", (1024, 1024), F32)
    din("wqk", (1024, 2048), BF16)
    din("wv", (1024, 1024), BF16)
    din("w_proj", (1024, 1024), F32)
    din("b_proj", (1024,), F32)
    din("w1", (1024, 4096), F32)
    din("b1t", (128, 32), F32)
    din("w2", (4096, 1024), F32)
    din("b2", (1024,), F32)
    din("masks", (128, 16, 512), BF16)
    if not ln1_triv:
        din("ln1_g", (1024,), F32)
        din("ln1_b", (1024,), F32)
    if not ln2_triv:
        din("ln2_g", (1024,), F32)
        din("ln2_b", (1024,), F32)
    io["out"] = nc.dram_tensor("out", [1024, 1024], F32, kind="ExternalOutput").ap()

    with tile.TileContext(nc) as tc:
        _emit_body(nc, tc, io, ln1_triv, ln2_triv)
    nc.compile()
    _CACHE[key] = (nc, io)
    return nc, io


def _chunks(half):
    if half == 0:
        return (0, 1536)   # chunk A base, chunk B base
    return (512, 1024)


def _make_masks(half):
    """[128, 16, 512] bf16: m 0-7 = slot0 tiles (queries=chunkA),
    m 8-15 = slot1 tiles 8-15 (queries=chunkB)."""
    qa, qb = _chunks(half)
    out = np.zeros((128, 16, 512), np.float32)
    tk_l = np.arange(128)[:, None]
    tq_l = np.arange(512)[None, :]
    for m in range(8):
        out[:, m, :] = ((m * 128 + tk_l) <= (qa + tq_l))
    for m in range(8, 16):
        out[:, m, :] = ((m * 128 + tk_l) <= (qb + tq_l))
    return out.astype(ml_dtypes.bfloat16)


def _prep_common(inp, ln1_triv, ln2_triv):
    wq_f = np.ascontiguousarray(inp["wq"].transpose(1, 0, 2).reshape(C, C))
    wk_f = np.ascontiguousarray(inp["wk"].transpose(1, 0, 2).reshape(C, C))
    wv_f = np.ascontiguousarray(inp["wv"].transpose(1, 0, 2).reshape(C, C))
    wqk = np.concatenate([wq_f, wk_f], axis=1).astype(ml_dtypes.bfloat16)
    b1t = np.ascontiguousarray(inp["b1"].reshape(32, 128).T).astype(np.float32)
    common = {
        "wqk": wqk,
        "wv": wv_f.astype(ml_dtypes.bfloat16),
        "w_proj": inp["w_proj"].astype(np.float32),
        "b_proj": inp["b_proj"].astype(np.float32),
        "w1": inp["w1"].astype(np.float32),
        "b1t": b1t,
        "w2": inp["w2"].astype(np.float32),
        "b2": inp["b2"].astype(np.float32),
    }
    if not ln1_triv:
        common["ln1_g"] = inp["ln1_g"].astype(np.float32)
        common["ln1_b"] = inp["ln1_b"].astype(np.float32)
    if not ln2_triv:
        common["ln2_g"] = inp["ln2_g"].astype(np.float32)
        common["ln2_b"] = inp["ln2_b"].astype(np.float32)
    return common


def make_in_maps(inputs):
    inp = {k: np.asarray(v) for k, v in inputs.items()}
    x = inp["x"].astype(np.float32)
    ln1_triv = bool(np.all(inp["ln1_g"] == 1.0) and np.all(inp["ln1_b"] == 0.0))
    ln2_triv = bool(np.all(inp["ln2_g"] == 1.0) and np.all(inp["ln2_b"] == 0.0))
    common = _prep_common(inp, ln1_triv, ln2_triv)
    in_maps = []
    for i in range(NCORES):
        b, half = i // 2, i % 2
        qa, qb = _chunks(half)
        xh = np.concatenate([x[b, qa:qa + 512], x[b, qb:qb + 512]], axis=0)
        m = dict(common)
        m["x_full"] = np.ascontiguousarray(x[b])
        m["x_half"] = np.ascontiguousarray(xh)
        m["masks"] = _make_masks(half)
        in_maps.append(m)
    return in_maps, ln1_triv, ln2_triv


def assemble(results):
    out = np.empty((B, T, C), np.float32)
    for i in range(NCORES):
        b, half = i // 2, i % 2
        qa, qb = _chunks(half)
        o = results[i]["out"]
        out[b, qa:qa + 512] = o[:512]
        out[b, qb:qb + 512] = o[512:]
    return out


def kernel(**inputs):
    in_maps, l1, l2 = make_in_maps(inputs)
    nc, io = build(l1, l2)
    res = run_bass_kernel_spmd(nc, in_maps, list(range(NCORES)))
    return assemble(res.results)


if __name__ == "__main__":
    build()
    print("build ok")
